# revision 7
# baseline (speedup 1.0000x reference)
"""BiBloSAN Trainium2 kernel (v2).

Shapes: B=4, N=16 blocks, R=64 tokens/block, D=256.
Sharding: one (batch, direction) pair per core -> 8 cores, no collectives.
The bw direction runs the SAME SPMD program on a host-reversed token
sequence (flat reverse maps the j<i mask onto the j>i program exactly).

v2 layout: full fp16 datapath (weights + activations), tight pair packing
(2176 pair-columns per (block, feature-half) instead of 2560), stacked
xi/xj K=128 select-matmuls, and folds rebalanced across DVE + Pool.

Pair-column layout per (block, dt): P = 2176 columns of (i, j) pairs.
 - cols [0, 256): 16 diagonal 4x4 blocks (chunk-major); for chunk c,
   sub-col (il, jl) is pair (i=4c+il, j=4c+jl); entries jl<=il are
   invalid and are zeroed with the m16 mask after exp.
 - cols [256, 2176): tails; chunk c (c=0..14) holds rows i=4c+il,
   j in [4c+4, 64), width jw'=60-4c, at offset TOFF[c].
Numerator/denominator folds: in-place fp16 halving adds per tail chunk
down to width <= 8, then a TensorReduce; the 4-wide diagonal part is a
single strided TensorReduce over all 16 chunks.
"""

import numpy as np
from contextlib import ExitStack

import concourse.bass as bass
import concourse.mybir as mybir
import concourse.tile as tile
from concourse import bacc, bass_utils

F32 = mybir.dt.float32
F16 = mybir.dt.float16
AF = mybir.ActivationFunctionType
ALU = mybir.AluOpType

B, NB, R, D = 4, 16, 64, 256
T = NB * R          # 1024 tokens
DT = D // 128       # 2 partition tiles of feature dim
C = 5.0
NCORES = 8
GB = 4              # blocks per instruction group
NGRP = NB // GB
ICH = 8             # i-rows per chunk
NCH = R // ICH      # 8 chunks
DIAGW = NCH * ICH * ICH       # 512 diagonal pair-cols
TJW = [56 - 8 * c for c in range(NCH - 1)]   # tail widths per chunk
TOFF = []
_o = DIAGW
for _c in range(NCH - 1):
    TOFF.append(_o)
    _o += ICH * TJW[_c]
P = _o              # 2304 pair-cols per (block, dt)
assert P == 2304


def _ap(t, offset, dims):
    """Raw AP on sbuf tile t: dims = [[step, count], ...] free dims."""
    base = t[:]
    return bass.AP(tensor=base.tensor, offset=base.offset + offset,
                   ap=[list(base.ap[0])] + [list(d) for d in dims])


def build_nc():
    nc = bacc.Bacc("TRN2", target_bir_lowering=False, debug=False,
                   num_devices=NCORES)

    # ---- DRAM I/O ----
    xT_d = nc.dram_tensor("xT", [D, T], F16, kind="ExternalInput").ap()
    w_d = {}
    for nm in ("fcW", "mW1", "mW2", "s2tW1", "s2tW", "gW1", "gW2"):
        w_d[nm] = nc.dram_tensor(nm, [D, D], F16, kind="ExternalInput").ap()
    for nm in ("fW1", "fW2"):
        w_d[nm] = nc.dram_tensor(nm, [3 * D, D], F16, kind="ExternalInput").ap()
    b_d = {}
    for nm in ("fcb", "mb", "s2tb1", "s2tb", "gb", "fb1", "fb2"):
        b_d[nm] = nc.dram_tensor(nm, [D], F32, kind="ExternalInput").ap()
    sela_d = nc.dram_tensor("sela", [128, P], F16, kind="ExternalInput").ap()
    m16_d = nc.dram_tensor("m16", [128, ICH * ICH], F16,
                           kind="ExternalInput").ap()
    eps64_d = nc.dram_tensor("eps64", [128, R], F16, kind="ExternalInput").ap()
    blkm_d = nc.dram_tensor("blkmask", [128, NB * NB], F32,
                            kind="ExternalInput").ap()
    eps16_d = nc.dram_tensor("eps16", [128, NB], F32, kind="ExternalInput").ap()
    out_d = nc.dram_tensor("outT", [D, 32], F32, kind="ExternalOutput").ap()

    with tile.TileContext(nc) as tc, ExitStack() as ctx:
        const = ctx.enter_context(tc.tile_pool(name="const", bufs=1))
        big = ctx.enter_context(tc.tile_pool(name="big", bufs=1))
        work = ctx.enter_context(tc.tile_pool(name="work", bufs=2))
        wpool = ctx.enter_context(tc.tile_pool(name="wpool", bufs=3))
        small = ctx.enter_context(tc.tile_pool(name="small", bufs=4))
        xijsb_pool = ctx.enter_context(tc.tile_pool(name="xijsb", bufs=6))
        psum = ctx.enter_context(
            tc.tile_pool(name="psum", bufs=1, space="PSUM"))
        xijps_pool = ctx.enter_context(
            tc.tile_pool(name="xijps", bufs=1, space="PSUM"))
        pairps_pool = ctx.enter_context(
            tc.tile_pool(name="pairps", bufs=2, space="PSUM"))

        # ---- load weights / constants (one DMA per tensor) ----
        wsb = {}
        def load_w(nm, nkt=2):
            t = const.tile([128, nkt * D], F16, tag=nm)
            nc.sync.dma_start(
                out=t[:].rearrange("p (kt e) -> p kt e", kt=nkt),
                in_=w_d[nm].rearrange("(kt p) e -> p kt e", p=128))
            wsb[nm] = t
        bsb = {}
        def load_b(nm):
            t = const.tile([128, DT], F32, tag=nm)
            nc.sync.dma_start(out=t[:],
                              in_=b_d[nm].rearrange("(dt p) -> p dt", p=128))
            bsb[nm] = t
        load_w("fcW")
        load_b("fcb")
        xT = big.tile([128, DT, T], F16, tag="xT")
        for hf in range(2):
            for dt in range(DT):
                nc.sync.dma_start(
                    out=xT[:, dt, hf * 512:(hf + 1) * 512],
                    in_=xT_d[dt * 128:(dt + 1) * 128, hf * 512:(hf + 1) * 512])
        for nm in ("mW1", "mW2"):
            load_w(nm)
        load_b("mb")
        mbC = const.tile([128, DT], F32)
        nc.scalar.mul(mbC[:], bsb["mb"][:], 1.0 / C)
        sela = const.tile([128, P], F16)
        nc.sync.dma_start(out=sela[:], in_=sela_d[:, :])
        m16 = const.tile([128, ICH * ICH], F16)
        nc.sync.dma_start(out=m16[:], in_=m16_d[:, :])
        eps64 = const.tile([128, R], F16)
        nc.sync.dma_start(out=eps64[:], in_=eps64_d[:, :])
        for nm in ("s2tb1", "s2tb", "gb", "fb1", "fb2"):
            load_b(nm)
        blkm = const.tile([128, NB * NB], F32)
        nc.sync.dma_start(out=blkm[:], in_=blkm_d[:, :])
        eps16 = const.tile([128, NB], F32)
        nc.sync.dma_start(out=eps16[:], in_=eps16_d[:, :])
        for nm in ("s2tW1", "s2tW", "gW1", "gW2"):
            load_w(nm)
        for nm in ("fW1", "fW2"):
            load_w(nm, nkt=6)

        # ---- FC: inp = relu(fcW.T @ xT + fcb), fp16 out ----
        inp = big.tile([128, DT, T], F16)
        for ncs in range(0, T, 512):
            for mt in range(DT):
                pt = psum.tile([128, 512], F32, tag="mmps")
                for kt in range(DT):
                    nc.tensor.matmul(
                        pt[:],
                        wsb["fcW"][:, kt * D + mt * 128: kt * D + (mt + 1) * 128],
                        xT[:, kt, ncs:ncs + 512],
                        start=(kt == 0), stop=(kt == DT - 1))
                nc.vector.tensor_scalar(inp[:, mt, ncs:ncs + 512], pt[:],
                                        bsb["fcb"][:, mt:mt + 1], 0.0,
                                        ALU.add, ALU.max)

        # ---- intra-block mSA ----
        ND = big.tile([128, DT, 2, T], F16, tag="ND")  # [...,0,:]=den [...,1,:]=num
        hT = big.tile([128, DT, T], F16, tag="hT")
        fT = big.tile([128, DT, T], F16, tag="fT")
        eT = big.tile([128, DT, T], F16, tag="eT")
        SUMS = small.tile([128, DT, NB], F32)
        NUMV = small.tile([128, DT, NB], F32)
        vT = small.tile([128, DT, NB], F16)
        rS = small.tile([128, DT, NB], F32, tag="rS")
        viT = small.tile([128, DT, NB], F32)
        vjT = small.tile([128, DT, NB], F32)

        # pair-matmul chunking of the P columns (psum-bank-sized windows)
        PCH = [(0, 1152), (1152, 1152)]

        for g in range(NGRP):
            # xi/xj for the 4 blocks, stacked on partitions (0-63 xi, 64-127 xj)
            xijs = []
            for bg in range(GB):
                tok0 = (g * GB + bg) * R
                xp = xijps_pool.tile([128, D], F32, tag="xijps")
                for kt in range(DT):
                    nc.tensor.matmul(
                        xp[0:64, :], inp[:, kt, tok0:tok0 + 64],
                        wsb["mW1"][:, kt * D:(kt + 1) * D],
                        start=(kt == 0), stop=(kt == DT - 1))
                for kt in range(DT):
                    nc.tensor.matmul(
                        xp[64:128, :], inp[:, kt, tok0:tok0 + 64],
                        wsb["mW2"][:, kt * D:(kt + 1) * D],
                        start=(kt == 0), stop=(kt == DT - 1))
                xsb = xijsb_pool.tile([128, D], F16, tag="xijsb")
                nc.scalar.copy(xsb[:], xp[:])
                xijs.append(xsb)

            for dt in range(DT):
                w16 = wpool.tile([128, GB, 2, P], F16, tag="w16")
                for bg in range(GB):
                    lhs = xijs[bg][:, dt * 128:(dt + 1) * 128]
                    for c0, cw in PCH:
                        pt = pairps_pool.tile([128, 1152], F32, tag="pairps")
                        for w0 in range(0, cw, 512):
                            ww = min(512, cw - w0)
                            nc.tensor.matmul(
                                pt[:, w0:w0 + ww], lhs,
                                sela[:, c0 + w0: c0 + w0 + ww],
                                start=True, stop=True)
                        nc.scalar.activation(
                            w16[:, bg, 0, c0:c0 + cw], pt[:, :cw], AF.Tanh,
                            bias=mbC[:, dt:dt + 1], scale=1.0 / C)
                # exp in place over the w halves, per block
                for bg in range(GB):
                    wall = _ap(w16, bg * 2 * P, [[1, P]])
                    nc.scalar.activation(wall, wall, AF.Exp, scale=C)

                # diagonal region: mask, then wx (Pool, per 2 blocks),
                # then halving folds (DVE)
                for bp in range(2):
                    o2 = bp * 2 * 2 * P
                    dmw = _ap(w16, o2, [[2 * P, 2], [ICH * ICH, NCH], [1, ICH * ICH]])
                    dm = _ap(m16, 0, [[0, 2], [0, NCH], [1, ICH * ICH]])
                    nc.gpsimd.tensor_mul(dmw, dmw, dm)
                    dwx = _ap(w16, o2 + P,
                              [[2 * P, 2], [ICH * ICH, NCH], [ICH, ICH], [1, ICH]])
                    dw = _ap(w16, o2,
                             [[2 * P, 2], [ICH * ICH, NCH], [ICH, ICH], [1, ICH]])
                    dx = _ap(inp, dt * T + (g * GB + bp * 2) * R,
                             [[R, 2], [ICH, NCH], [0, ICH], [1, ICH]])
                    nc.gpsimd.tensor_mul(dwx, dw, dx)
                # diag fold: 8 -> 4 -> 2 -> 1 (into ND)
                da0 = _ap(w16, 0, [[2 * P, GB], [P, 2], [ICH, R], [1, 4]])
                da1 = _ap(w16, 4, [[2 * P, GB], [P, 2], [ICH, R], [1, 4]])
                nc.vector.tensor_add(da0, da0, da1)
                db0 = _ap(w16, 0, [[2 * P, GB], [P, 2], [ICH, R], [1, 2]])
                db1 = _ap(w16, 2, [[2 * P, GB], [P, 2], [ICH, R], [1, 2]])
                nc.vector.tensor_add(db0, db0, db1)
                ndo = _ap(ND, dt * 2 * T + g * GB * R,
                          [[R, GB], [T, 2], [1, R]])
                dc0 = _ap(w16, 0, [[2 * P, GB], [P, 2], [ICH, R]])
                dc1 = _ap(w16, 1, [[2 * P, GB], [P, 2], [ICH, R]])
                nc.vector.tensor_add(ndo, dc0, dc1)

                # tails: wx mul, uneven in-place halving folds into TSUM
                TSUM = small.tile([128, GB, 2, R], F16, tag="tsum")
                for c in range(NCH - 1):
                    jwp = TJW[c]
                    toff = TOFF[c]
                    wv = _ap(w16, toff, [[2 * P, GB], [jwp, ICH], [1, jwp]])
                    wxv = _ap(w16, P + toff,
                              [[2 * P, GB], [jwp, ICH], [1, jwp]])
                    xv = _ap(inp, dt * T + g * GB * R + ICH * c + ICH,
                             [[R, GB], [0, ICH], [1, jwp]])
                    nc.vector.tensor_mul(wxv, wv, xv)
                    h = jwp
                    first = True
                    while h > 2:
                        k = h // 2
                        hp = h - k      # ceil
                        a0 = _ap(w16, toff,
                                 [[2 * P, GB], [P, 2], [jwp, ICH], [1, k]])
                        a1 = _ap(w16, toff + hp,
                                 [[2 * P, GB], [P, 2], [jwp, ICH], [1, k]])
                        if first and c == 0:
                            nc.gpsimd.tensor_add(a0, a0, a1)
                        else:
                            nc.vector.tensor_add(a0, a0, a1)
                        first = False
                        h = hp
                    tout = _ap(TSUM, ICH * c,
                               [[2 * R, GB], [R, 2], [1, ICH]])
                    t0 = _ap(w16, toff, [[2 * P, GB], [P, 2], [jwp, ICH]])
                    t1 = _ap(w16, toff + 1, [[2 * P, GB], [P, 2], [jwp, ICH]])
                    nc.vector.tensor_add(tout, t0, t1)
                # ND rows 0..55 += TSUM; den += eps (row 63)
                nda = _ap(ND, dt * 2 * T + g * GB * R,
                          [[R, GB], [T, 2], [1, R - ICH]])
                tsa = _ap(TSUM, 0, [[2 * R, GB], [R, 2], [1, R - ICH]])
                nc.gpsimd.tensor_add(nda, nda, tsa)
                epsa = _ap(ND, dt * 2 * T + g * GB * R, [[R, GB], [1, R]])
                nc.gpsimd.tensor_add(epsa, epsa,
                                     _ap(eps64, 0, [[0, GB], [1, R]]))

                # h = num / den  (fp16)
                g0 = g * GB * R
                den = ND[:, dt, 0, g0:g0 + GB * R]
                num = ND[:, dt, 1, g0:g0 + GB * R]
                with nc.allow_low_precision("fp16 softmax normalize"):
                    nc.vector.reciprocal(den, den)
                nc.vector.tensor_mul(hT[:, dt, g0:g0 + GB * R], num, den)

            # ---- s2t for this group's 4 blocks ----
            GC = GB * R
            g0 = g * GC
            for mt in range(DT):
                ptf = psum.tile([128, GC], F32, tag="mmps")
                for kt in range(DT):
                    nc.tensor.matmul(
                        ptf[:],
                        wsb["s2tW1"][:, kt * D + mt * 128: kt * D + (mt + 1) * 128],
                        hT[:, kt, g0:g0 + GC], start=(kt == 0),
                        stop=(kt == DT - 1))
                nc.vector.tensor_scalar(fT[:, mt, g0:g0 + GC], ptf[:],
                                        bsb["s2tb1"][:, mt:mt + 1], 0.0,
                                        ALU.add, ALU.max)
            for mt in range(DT):
                pte = psum.tile([128, GC], F32, tag="mmps")
                for kt in range(DT):
                    nc.tensor.matmul(
                        pte[:],
                        wsb["s2tW"][:, kt * D + mt * 128: kt * D + (mt + 1) * 128],
                        fT[:, kt, g0:g0 + GC], start=(kt == 0),
                        stop=(kt == DT - 1))
                nc.scalar.activation(eT[:, mt, g0:g0 + GC], pte[:], AF.Exp,
                                     bias=bsb["s2tb"][:, mt:mt + 1])
            for dt in range(DT):
                nc.vector.tensor_reduce(
                    SUMS[:, dt, g * GB:(g + 1) * GB],
                    eT[:, dt, g0:g0 + GC].rearrange("p (n r) -> p n r", r=R),
                    mybir.AxisListType.X, ALU.add)
                wh = work.tile([128, GC], F16, tag="wh")
                nc.vector.tensor_mul(wh[:], eT[:, dt, g0:g0 + GC],
                                     hT[:, dt, g0:g0 + GC])
                nc.vector.tensor_reduce(
                    NUMV[:, dt, g * GB:(g + 1) * GB],
                    wh[:].rearrange("p (n r) -> p n r", r=R),
                    mybir.AxisListType.X, ALU.add)
            gB = g * GB
            for dt in range(DT):
                nc.vector.reciprocal(rS[:, dt, gB:gB + GB],
                                     SUMS[:, dt, gB:gB + GB])
                nc.vector.tensor_mul(vT[:, dt, gB:gB + GB],
                                     NUMV[:, dt, gB:gB + GB],
                                     rS[:, dt, gB:gB + GB])
            for dst, wname in ((viT, "mW1"), (vjT, "mW2")):
                w = wsb[wname]
                for mt in range(DT):
                    pt = psum.tile([128, GB], F32, tag="mmps")
                    for kt in range(DT):
                        nc.tensor.matmul(
                            pt[:],
                            w[:, kt * D + mt * 128: kt * D + (mt + 1) * 128],
                            vT[:, kt, gB:gB + GB], start=(kt == 0),
                            stop=(kt == DT - 1))
                    nc.vector.tensor_copy(dst[:, mt, gB:gB + GB], pt[:])
        oT = small.tile([128, DT, NB], F32)
        ub = work.tile([128, DT, NB, NB], F32, tag="ublk")
        vi2 = _ap(viT, 0, [[NB, DT], [1, NB], [0, NB]])
        vj2 = _ap(vjT, 0, [[NB, DT], [0, NB], [1, NB]])
        nc.vector.tensor_add(ub[:], vi2, vj2)
        for dt in range(DT):
            nc.scalar.activation(ub[:, dt], ub[:, dt], AF.Tanh,
                                 bias=mbC[:, dt:dt + 1], scale=1.0 / C)
        nc.scalar.activation(ub[:], ub[:], AF.Exp, scale=C)
        bm = _ap(blkm, 0, [[0, DT], [NB, NB], [1, NB]])
        nc.vector.tensor_mul(ub[:], ub[:], bm)
        deno = small.tile([128, DT, NB], F32, tag="deno")
        nc.vector.tensor_reduce(deno[:], ub[:], mybir.AxisListType.X,
                                ALU.add)
        nc.vector.tensor_add(deno[:], deno[:],
                             _ap(eps16, 0, [[0, DT], [1, NB]]))
        wv2 = work.tile([128, DT, NB, NB], F32, tag="wv2")
        vTf = small.tile([128, DT, NB], F32, tag="vTf")
        for dt in range(DT):
            nc.vector.tensor_copy(vTf[:, dt, :], vT[:, dt, :])
        nc.vector.tensor_mul(wv2[:], ub[:],
                             _ap(vTf, 0, [[NB, DT], [0, NB], [1, NB]]))
        numo = small.tile([128, DT, NB], F32, tag="numo")
        nc.vector.tensor_reduce(numo[:], wv2[:], mybir.AxisListType.X,
                                ALU.add)
        nc.vector.reciprocal(deno[:], deno[:])
        nc.vector.tensor_mul(oT[:], numo[:], deno[:])

        # ---- gating at rows 0 and 15 (fp16 operands for the matmuls) ----
        o01 = small.tile([128, DT, 2], F16)
        v01 = small.tile([128, DT, 2], F16)
        for dt in range(DT):
            nc.vector.tensor_copy(o01[:, dt, :],
                                  _ap(oT, dt * NB, [[NB - 1, 2]]))
            nc.vector.tensor_copy(v01[:, dt, :],
                                  _ap(vTf, dt * NB, [[NB - 1, 2]]))
        G01 = small.tile([128, DT, 2], F32)
        for mt in range(DT):
            pt = psum.tile([128, 2], F32, tag="mmps")
            for kt in range(DT):
                nc.tensor.matmul(
                    pt[:], wsb["gW1"][:, kt * D + mt * 128: kt * D + (mt + 1) * 128],
                    o01[:, kt, :], start=(kt == 0), stop=False)
            for kt in range(DT):
                nc.tensor.matmul(
                    pt[:], wsb["gW2"][:, kt * D + mt * 128: kt * D + (mt + 1) * 128],
                    v01[:, kt, :], start=False, stop=(kt == DT - 1))
            nc.scalar.activation(G01[:, mt, :], pt[:], AF.Sigmoid,
                                 bias=bsb["gb"][:, mt:mt + 1])
        e01 = small.tile([128, DT, 2], F32)
        for dt in range(DT):
            tmp = small.tile([128, 2], F32, tag="etmp")
            nc.vector.tensor_sub(tmp[:], o01[:, dt, :], v01[:, dt, :])
            nc.vector.tensor_mul(tmp[:], tmp[:], G01[:, dt, :])
            nc.vector.tensor_add(e01[:, dt, :], v01[:, dt, :], tmp[:])

        # ---- fusion for both candidate slices ----
        # slice A: cols 0:16 with E=e01[...,0]; slice B: cols 1008:1024, E=e01[...,1]
        EA = small.tile([128, DT, 2, 16], F16)   # [dt, slice, 16]
        for dt in range(DT):
            for s in range(2):
                nc.vector.tensor_copy(EA[:, dt, s, :],
                                      _ap(e01, dt * 2 + s, [[0, 16]]))
        outT = small.tile([128, DT, 32], F32)
        scol = (0, T - 16)
        for wname, bname, func, dstname in (("fW1", "fb1", AF.Relu, "fus"),
                                            ("fW2", "fb2", AF.Sigmoid, "gf")):
            dst = small.tile([128, DT, 32], F32, tag=dstname)
            if dstname == "fus":
                fus = dst
            else:
                gf = dst
            for mt in range(DT):
                for s in range(2):
                    c0 = scol[s]
                    pt = psum.tile([128, 16], F32, tag="mmps")
                    for kt in range(6):
                        if kt < 2:
                            rhs = inp[:, kt, c0:c0 + 16]
                        elif kt < 4:
                            rhs = hT[:, kt - 2, c0:c0 + 16]
                        else:
                            rhs = EA[:, kt - 4, s, :]
                        nc.tensor.matmul(
                            pt[:],
                            wsb[wname][:, kt * D + mt * 128: kt * D + (mt + 1) * 128],
                            rhs, start=(kt == 0), stop=(kt == 5))
                    nc.scalar.activation(dst[:, mt, s * 16:(s + 1) * 16], pt[:],
                                         func, bias=bsb[bname][:, mt:mt + 1])
        for mt in range(DT):
            for s in range(2):
                xf = inp[:, mt, scol[s]:scol[s] + 16]
                of = outT[:, mt, s * 16:(s + 1) * 16]
                nc.vector.tensor_sub(of, fus[:, mt, s * 16:(s + 1) * 16], xf)
                nc.vector.tensor_mul(of, of, gf[:, mt, s * 16:(s + 1) * 16])
                nc.vector.tensor_add(of, of, xf)
        for mt in range(DT):
            nc.sync.dma_start(out=out_d[mt * 128:(mt + 1) * 128, :],
                              in_=outT[:, mt, :])
    nc.compile()
    return nc


_NC = None


def _get_nc():
    global _NC
    if _NC is None:
        _NC = build_nc()
    return _NC


def _consts():
    il = np.arange(ICH)
    m = (il[None, :] > il[:, None]).astype(np.float16).reshape(-1)
    m16 = np.broadcast_to(m, (128, ICH * ICH)).copy()
    bi = np.arange(NB)
    blk = (bi[None, :] > bi[:, None]).astype(np.float32).reshape(-1)
    blkmask = np.broadcast_to(blk, (128, NB * NB)).copy()
    e64 = np.zeros(R, np.float16); e64[R - 1] = 1.0
    eps64 = np.broadcast_to(e64, (128, R)).copy()
    e16 = np.zeros(NB, np.float32); e16[NB - 1] = 1.0
    eps16 = np.broadcast_to(e16, (128, NB)).copy()
    selI = np.zeros((64, P), np.float16)
    selJ = np.zeros((64, P), np.float16)
    col = 0
    for c in range(NCH):                       # diagonal 4x4 blocks
        for il_ in range(ICH):
            for jl in range(ICH):
                selI[ICH * c + il_, col] = 1.0
                selJ[ICH * c + jl, col] = 1.0
                col += 1
    assert col == DIAGW
    for c in range(NCH - 1):                   # tails
        for il_ in range(ICH):
            for j in range(ICH * c + ICH, R):
                selI[ICH * c + il_, col] = 1.0
                selJ[j, col] = 1.0
                col += 1
    assert col == P
    sela = np.concatenate([selI, selJ], 0)
    return m16, blkmask, eps64, eps16, sela


def prep_in_maps(inputs):
    x = np.asarray(inputs["x"], np.float32)
    m16, blkmask, eps64, eps16, sela = _consts()
    wnames = ("fcW", "mW1", "mW2", "s2tW1", "s2tW", "gW1", "gW2", "fW1", "fW2")
    bnames = ("fcb", "mb", "s2tb1", "s2tb", "gb", "fb1", "fb2")

    in_maps = []
    for core in range(NCORES):
        b = core % B
        sfx = "_fw" if core < B else "_bw"
        xf = x[b].reshape(T, D)
        if core >= B:
            xf = xf[::-1]
        m = {"xT": np.ascontiguousarray(xf.T.astype(np.float16)),
             "m16": m16, "blkmask": blkmask,
             "eps64": eps64, "eps16": eps16, "sela": sela}
        for nm in wnames:
            m[nm] = np.ascontiguousarray(
                np.asarray(inputs[nm + sfx]).astype(np.float16))
        for nm in bnames:
            m[nm] = np.ascontiguousarray(
                np.asarray(inputs[nm + sfx]).astype(np.float32))
        in_maps.append(m)
    return in_maps


def assemble(outs):
    u_fw = np.stack([outs[b]["outT"][:, 0:16].T for b in range(B)])
    u_bw = np.stack([outs[B + b]["outT"][:, 16:32].T[::-1] for b in range(B)])
    return np.concatenate([u_fw, u_bw], axis=-1).astype(np.float32)


def kernel(**inputs):
    in_maps = prep_in_maps(inputs)
    res = bass_utils.run_bass_kernel_spmd(_get_nc(), in_maps,
                                          core_ids=list(range(NCORES)))
    return assemble(res.results)


# revision 9
# speedup vs baseline: 1.0513x; 1.0513x over previous
"""BiBloSAN Trainium2 kernel (v2).

Shapes: B=4, N=16 blocks, R=64 tokens/block, D=256.
Sharding: one (batch, direction) pair per core -> 8 cores, no collectives.
The bw direction runs the SAME SPMD program on a host-reversed token
sequence (flat reverse maps the j<i mask onto the j>i program exactly).

v2 layout: full fp16 datapath (weights + activations), tight pair packing
(2176 pair-columns per (block, feature-half) instead of 2560), stacked
xi/xj K=128 select-matmuls, and folds rebalanced across DVE + Pool.

Pair-column layout per (block, dt): P = 2176 columns of (i, j) pairs.
 - cols [0, 256): 16 diagonal 4x4 blocks (chunk-major); for chunk c,
   sub-col (il, jl) is pair (i=4c+il, j=4c+jl); entries jl<=il are
   invalid and are zeroed with the m16 mask after exp.
 - cols [256, 2176): tails; chunk c (c=0..14) holds rows i=4c+il,
   j in [4c+4, 64), width jw'=60-4c, at offset TOFF[c].
Numerator/denominator folds: in-place fp16 halving adds per tail chunk
down to width <= 8, then a TensorReduce; the 4-wide diagonal part is a
single strided TensorReduce over all 16 chunks.
"""

import numpy as np
from contextlib import ExitStack

import concourse.bass as bass
import concourse.mybir as mybir
import concourse.tile as tile
from concourse import bacc, bass_utils

F32 = mybir.dt.float32
F16 = mybir.dt.float16
AF = mybir.ActivationFunctionType
ALU = mybir.AluOpType

B, NB, R, D = 4, 16, 64, 256
T = NB * R          # 1024 tokens
DT = D // 128       # 2 partition tiles of feature dim
C = 5.0
NCORES = 8
GB = 4              # blocks per instruction group
NGRP = NB // GB
ICH = 8             # i-rows per chunk
NCH = R // ICH      # 8 chunks
DIAGW = NCH * ICH * ICH       # 512 diagonal pair-cols
TJW = [56 - 8 * c for c in range(NCH - 1)]   # tail widths per chunk
TOFF = []
_o = DIAGW
for _c in range(NCH - 1):
    TOFF.append(_o)
    _o += ICH * TJW[_c]
P = _o              # 2304 pair-cols per (block, dt)
assert P == 2304


def _ap(t, offset, dims):
    """Raw AP on sbuf tile t: dims = [[step, count], ...] free dims."""
    base = t[:]
    return bass.AP(tensor=base.tensor, offset=base.offset + offset,
                   ap=[list(base.ap[0])] + [list(d) for d in dims])


def build_nc():
    nc = bacc.Bacc("TRN2", target_bir_lowering=False, debug=False,
                   num_devices=NCORES)

    # ---- DRAM I/O ----
    xT_d = nc.dram_tensor("xT", [D, T], F16, kind="ExternalInput").ap()
    w_d = {}
    for nm in ("fcW", "mW1", "mW2", "s2tW1", "s2tW", "gW1", "gW2"):
        w_d[nm] = nc.dram_tensor(nm, [D, D], F16, kind="ExternalInput").ap()
    for nm in ("fW1", "fW2"):
        w_d[nm] = nc.dram_tensor(nm, [3 * D, D], F16, kind="ExternalInput").ap()
    b_d = {}
    for nm in ("fcb", "mb", "s2tb1", "s2tb", "gb", "fb1", "fb2"):
        b_d[nm] = nc.dram_tensor(nm, [D], F32, kind="ExternalInput").ap()
    sela_d = nc.dram_tensor("sela", [128, P], F16, kind="ExternalInput").ap()
    m16_d = nc.dram_tensor("m16", [128, ICH * ICH], F16,
                           kind="ExternalInput").ap()
    eps64_d = nc.dram_tensor("eps64", [128, R], F16, kind="ExternalInput").ap()
    blkm_d = nc.dram_tensor("blkmask", [128, NB * NB], F32,
                            kind="ExternalInput").ap()
    eps16_d = nc.dram_tensor("eps16", [128, NB], F32, kind="ExternalInput").ap()
    out_d = nc.dram_tensor("outT", [D, 32], F32, kind="ExternalOutput").ap()

    with tile.TileContext(nc) as tc, ExitStack() as ctx:
        const = ctx.enter_context(tc.tile_pool(name="const", bufs=1))
        big = ctx.enter_context(tc.tile_pool(name="big", bufs=1))
        work = ctx.enter_context(tc.tile_pool(name="work", bufs=2))
        wpool = ctx.enter_context(tc.tile_pool(name="wpool", bufs=3))
        small = ctx.enter_context(tc.tile_pool(name="small", bufs=4))
        xijsb_pool = ctx.enter_context(tc.tile_pool(name="xijsb", bufs=6))
        psum = ctx.enter_context(
            tc.tile_pool(name="psum", bufs=1, space="PSUM"))
        xijps_pool = ctx.enter_context(
            tc.tile_pool(name="xijps", bufs=1, space="PSUM"))
        pairps_pool = ctx.enter_context(
            tc.tile_pool(name="pairps", bufs=2, space="PSUM"))

        # ---- load weights / constants (one DMA per tensor) ----
        wsb = {}
        def load_w(nm, nkt=2):
            t = const.tile([128, nkt * D], F16, tag=nm)
            nc.sync.dma_start(
                out=t[:].rearrange("p (kt e) -> p kt e", kt=nkt),
                in_=w_d[nm].rearrange("(kt p) e -> p kt e", p=128))
            wsb[nm] = t
        bsb = {}
        def load_b(nm):
            t = const.tile([128, DT], F32, tag=nm)
            nc.sync.dma_start(out=t[:],
                              in_=b_d[nm].rearrange("(dt p) -> p dt", p=128))
            bsb[nm] = t
        load_w("fcW")
        load_b("fcb")
        xT = big.tile([128, DT, T], F16, tag="xT")
        for hf in range(2):
            for dt in range(DT):
                nc.sync.dma_start(
                    out=xT[:, dt, hf * 512:(hf + 1) * 512],
                    in_=xT_d[dt * 128:(dt + 1) * 128, hf * 512:(hf + 1) * 512])
        for nm in ("mW1", "mW2"):
            load_w(nm)
        load_b("mb")
        mbC = const.tile([128, DT], F32)
        nc.scalar.mul(mbC[:], bsb["mb"][:], 1.0 / C)
        sela = const.tile([128, P], F16)
        nc.sync.dma_start(out=sela[:], in_=sela_d[:, :])
        m16 = const.tile([128, ICH * ICH], F16)
        nc.sync.dma_start(out=m16[:], in_=m16_d[:, :])
        eps64 = const.tile([128, R], F16)
        nc.sync.dma_start(out=eps64[:], in_=eps64_d[:, :])
        for nm in ("s2tb1", "s2tb", "gb", "fb1", "fb2"):
            load_b(nm)
        blkm = const.tile([128, NB * NB], F32)
        nc.sync.dma_start(out=blkm[:], in_=blkm_d[:, :])
        eps16 = const.tile([128, NB], F32)
        nc.sync.dma_start(out=eps16[:], in_=eps16_d[:, :])
        for nm in ("s2tW1", "s2tW", "gW1", "gW2"):
            load_w(nm)
        for nm in ("fW1", "fW2"):
            load_w(nm, nkt=6)

        # ---- FC: inp = relu(fcW.T @ xT + fcb), fp16 out ----
        inp = big.tile([128, DT, T], F16)
        for ncs in range(0, T, 512):
            for mt in range(DT):
                pt = psum.tile([128, 512], F32, tag="mmps")
                for kt in range(DT):
                    nc.tensor.matmul(
                        pt[:],
                        wsb["fcW"][:, kt * D + mt * 128: kt * D + (mt + 1) * 128],
                        xT[:, kt, ncs:ncs + 512],
                        start=(kt == 0), stop=(kt == DT - 1))
                nc.vector.tensor_scalar(inp[:, mt, ncs:ncs + 512], pt[:],
                                        bsb["fcb"][:, mt:mt + 1], 0.0,
                                        ALU.add, ALU.max)

        # ---- intra-block mSA ----
        ND = big.tile([128, DT, 2, T], F16, tag="ND")  # [...,0,:]=den [...,1,:]=num
        hT = big.tile([128, DT, T], F16, tag="hT")
        fT = big.tile([128, DT, T], F16, tag="fT")
        eT = big.tile([128, DT, T], F16, tag="eT")
        SUMS = small.tile([128, DT, NB], F32)
        NUMV = small.tile([128, DT, NB], F32)
        vT = small.tile([128, DT, NB], F16)
        rS = small.tile([128, DT, NB], F32, tag="rS")
        viT = small.tile([128, DT, NB], F32)
        vjT = small.tile([128, DT, NB], F32)

        # pair-matmul chunking of the P columns (psum-bank-sized windows)
        PCH = [(0, 1152), (1152, 1152)]

        def msa_group(g):
            # xi/xj for the 4 blocks, stacked on partitions (0-63 xi, 64-127 xj)
            xijs = []
            for bg in range(GB):
                tok0 = (g * GB + bg) * R
                xp = xijps_pool.tile([128, D], F32, tag="xijps")
                for kt in range(DT):
                    nc.tensor.matmul(
                        xp[0:64, :], inp[:, kt, tok0:tok0 + 64],
                        wsb["mW1"][:, kt * D:(kt + 1) * D],
                        start=(kt == 0), stop=(kt == DT - 1))
                for kt in range(DT):
                    nc.tensor.matmul(
                        xp[64:128, :], inp[:, kt, tok0:tok0 + 64],
                        wsb["mW2"][:, kt * D:(kt + 1) * D],
                        start=(kt == 0), stop=(kt == DT - 1))
                xsb = xijsb_pool.tile([128, D], F16, tag="xijsb")
                nc.scalar.copy(xsb[:], xp[:])
                xijs.append(xsb)

            for dt in range(DT):
                w16 = wpool.tile([128, GB, 2, P], F16, tag="w16")
                for bg in range(GB):
                    lhs = xijs[bg][:, dt * 128:(dt + 1) * 128]
                    for c0, cw in PCH:
                        pt = pairps_pool.tile([128, 1152], F32, tag="pairps")
                        for w0 in range(0, cw, 512):
                            ww = min(512, cw - w0)
                            nc.tensor.matmul(
                                pt[:, w0:w0 + ww], lhs,
                                sela[:, c0 + w0: c0 + w0 + ww],
                                start=True, stop=True)
                        nc.scalar.activation(
                            w16[:, bg, 0, c0:c0 + cw], pt[:, :cw], AF.Tanh,
                            bias=mbC[:, dt:dt + 1], scale=1.0 / C)
                # exp in place over the w halves, per block
                for bg in range(GB):
                    wall = _ap(w16, bg * 2 * P, [[1, P]])
                    nc.scalar.activation(wall, wall, AF.Exp, scale=C)

                # diagonal region: mask, then wx (Pool, per 2 blocks),
                # then halving folds (DVE)
                for bp in range(2):
                    o2 = bp * 2 * 2 * P
                    dmw = _ap(w16, o2, [[2 * P, 2], [ICH * ICH, NCH], [1, ICH * ICH]])
                    dm = _ap(m16, 0, [[0, 2], [0, NCH], [1, ICH * ICH]])
                    nc.gpsimd.tensor_mul(dmw, dmw, dm)
                    dwx = _ap(w16, o2 + P,
                              [[2 * P, 2], [ICH * ICH, NCH], [ICH, ICH], [1, ICH]])
                    dw = _ap(w16, o2,
                             [[2 * P, 2], [ICH * ICH, NCH], [ICH, ICH], [1, ICH]])
                    dx = _ap(inp, dt * T + (g * GB + bp * 2) * R,
                             [[R, 2], [ICH, NCH], [0, ICH], [1, ICH]])
                    nc.gpsimd.tensor_mul(dwx, dw, dx)
                # diag fold: 8 -> 4 -> 2 -> 1 (into ND)
                da0 = _ap(w16, 0, [[2 * P, GB], [P, 2], [ICH, R], [1, 4]])
                da1 = _ap(w16, 4, [[2 * P, GB], [P, 2], [ICH, R], [1, 4]])
                nc.vector.tensor_add(da0, da0, da1)
                db0 = _ap(w16, 0, [[2 * P, GB], [P, 2], [ICH, R], [1, 2]])
                db1 = _ap(w16, 2, [[2 * P, GB], [P, 2], [ICH, R], [1, 2]])
                nc.vector.tensor_add(db0, db0, db1)
                ndo = _ap(ND, dt * 2 * T + g * GB * R,
                          [[R, GB], [T, 2], [1, R]])
                dc0 = _ap(w16, 0, [[2 * P, GB], [P, 2], [ICH, R]])
                dc1 = _ap(w16, 1, [[2 * P, GB], [P, 2], [ICH, R]])
                nc.vector.tensor_add(ndo, dc0, dc1)

                # tails: wx mul, uneven in-place halving folds into TSUM
                TSUM = small.tile([128, GB, 2, R], F16, tag="tsum")
                for c in range(NCH - 1):
                    jwp = TJW[c]
                    toff = TOFF[c]
                    wv = _ap(w16, toff, [[2 * P, GB], [jwp, ICH], [1, jwp]])
                    wxv = _ap(w16, P + toff,
                              [[2 * P, GB], [jwp, ICH], [1, jwp]])
                    xv = _ap(inp, dt * T + g * GB * R + ICH * c + ICH,
                             [[R, GB], [0, ICH], [1, jwp]])
                    nc.vector.tensor_mul(wxv, wv, xv)
                    h = jwp
                    first = True
                    while h > 2:
                        k = h // 2
                        hp = h - k      # ceil
                        a0 = _ap(w16, toff,
                                 [[2 * P, GB], [P, 2], [jwp, ICH], [1, k]])
                        a1 = _ap(w16, toff + hp,
                                 [[2 * P, GB], [P, 2], [jwp, ICH], [1, k]])
                        if first and c == 0:
                            nc.gpsimd.tensor_add(a0, a0, a1)
                        else:
                            nc.vector.tensor_add(a0, a0, a1)
                        first = False
                        h = hp
                    tout = _ap(TSUM, ICH * c,
                               [[2 * R, GB], [R, 2], [1, ICH]])
                    t0 = _ap(w16, toff, [[2 * P, GB], [P, 2], [jwp, ICH]])
                    t1 = _ap(w16, toff + 1, [[2 * P, GB], [P, 2], [jwp, ICH]])
                    nc.vector.tensor_add(tout, t0, t1)
                # ND rows 0..55 += TSUM; den += eps (row 63)
                nda = _ap(ND, dt * 2 * T + g * GB * R,
                          [[R, GB], [T, 2], [1, R - ICH]])
                tsa = _ap(TSUM, 0, [[2 * R, GB], [R, 2], [1, R - ICH]])
                nc.gpsimd.tensor_add(nda, nda, tsa)
                epsa = _ap(ND, dt * 2 * T + g * GB * R, [[R, GB], [1, R]])
                nc.gpsimd.tensor_add(epsa, epsa,
                                     _ap(eps64, 0, [[0, GB], [1, R]]))

                # h = num / den  (fp16)
                g0 = g * GB * R
                den = ND[:, dt, 0, g0:g0 + GB * R]
                num = ND[:, dt, 1, g0:g0 + GB * R]
                with nc.allow_low_precision("fp16 softmax normalize"):
                    nc.vector.reciprocal(den, den)
                nc.vector.tensor_mul(hT[:, dt, g0:g0 + GB * R], num, den)

        def s2t_group(g):
            # ---- s2t for group g's 4 blocks (runs one group behind) ----
            GC = GB * R
            g0 = g * GC
            for mt in range(DT):
                ptf = psum.tile([128, GC], F32, tag="mmps")
                for kt in range(DT):
                    nc.tensor.matmul(
                        ptf[:],
                        wsb["s2tW1"][:, kt * D + mt * 128: kt * D + (mt + 1) * 128],
                        hT[:, kt, g0:g0 + GC], start=(kt == 0),
                        stop=(kt == DT - 1))
                nc.vector.tensor_scalar(fT[:, mt, g0:g0 + GC], ptf[:],
                                        bsb["s2tb1"][:, mt:mt + 1], 0.0,
                                        ALU.add, ALU.max)
            for mt in range(DT):
                pte = psum.tile([128, GC], F32, tag="mmps")
                for kt in range(DT):
                    nc.tensor.matmul(
                        pte[:],
                        wsb["s2tW"][:, kt * D + mt * 128: kt * D + (mt + 1) * 128],
                        fT[:, kt, g0:g0 + GC], start=(kt == 0),
                        stop=(kt == DT - 1))
                nc.scalar.activation(eT[:, mt, g0:g0 + GC], pte[:], AF.Exp,
                                     bias=bsb["s2tb"][:, mt:mt + 1])
            for dt in range(DT):
                nc.vector.tensor_reduce(
                    SUMS[:, dt, g * GB:(g + 1) * GB],
                    eT[:, dt, g0:g0 + GC].rearrange("p (n r) -> p n r", r=R),
                    mybir.AxisListType.X, ALU.add)
                wh = work.tile([128, GC], F16, tag="wh")
                nc.vector.tensor_mul(wh[:], eT[:, dt, g0:g0 + GC],
                                     hT[:, dt, g0:g0 + GC])
                nc.vector.tensor_reduce(
                    NUMV[:, dt, g * GB:(g + 1) * GB],
                    wh[:].rearrange("p (n r) -> p n r", r=R),
                    mybir.AxisListType.X, ALU.add)
            gB = g * GB
            for dt in range(DT):
                nc.vector.reciprocal(rS[:, dt, gB:gB + GB],
                                     SUMS[:, dt, gB:gB + GB])
                nc.vector.tensor_mul(vT[:, dt, gB:gB + GB],
                                     NUMV[:, dt, gB:gB + GB],
                                     rS[:, dt, gB:gB + GB])
            for dst, wname in ((viT, "mW1"), (vjT, "mW2")):
                w = wsb[wname]
                for mt in range(DT):
                    pt = psum.tile([128, GB], F32, tag="mmps")
                    for kt in range(DT):
                        nc.tensor.matmul(
                            pt[:],
                            w[:, kt * D + mt * 128: kt * D + (mt + 1) * 128],
                            vT[:, kt, gB:gB + GB], start=(kt == 0),
                            stop=(kt == DT - 1))
                    nc.vector.tensor_copy(dst[:, mt, gB:gB + GB], pt[:])

        for g in range(NGRP):
            msa_group(g)
            if g >= 1:
                s2t_group(g - 1)
        s2t_group(NGRP - 1)

        oT = small.tile([128, DT, NB], F32)
        ub = work.tile([128, DT, NB, NB], F32, tag="ublk")
        vi2 = _ap(viT, 0, [[NB, DT], [1, NB], [0, NB]])
        vj2 = _ap(vjT, 0, [[NB, DT], [0, NB], [1, NB]])
        nc.vector.tensor_add(ub[:], vi2, vj2)
        for dt in range(DT):
            nc.scalar.activation(ub[:, dt], ub[:, dt], AF.Tanh,
                                 bias=mbC[:, dt:dt + 1], scale=1.0 / C)
        nc.scalar.activation(ub[:], ub[:], AF.Exp, scale=C)
        bm = _ap(blkm, 0, [[0, DT], [NB, NB], [1, NB]])
        nc.vector.tensor_mul(ub[:], ub[:], bm)
        deno = small.tile([128, DT, NB], F32, tag="deno")
        nc.vector.tensor_reduce(deno[:], ub[:], mybir.AxisListType.X,
                                ALU.add)
        nc.vector.tensor_add(deno[:], deno[:],
                             _ap(eps16, 0, [[0, DT], [1, NB]]))
        wv2 = work.tile([128, DT, NB, NB], F32, tag="wv2")
        vTf = small.tile([128, DT, NB], F32, tag="vTf")
        for dt in range(DT):
            nc.vector.tensor_copy(vTf[:, dt, :], vT[:, dt, :])
        nc.vector.tensor_mul(wv2[:], ub[:],
                             _ap(vTf, 0, [[NB, DT], [0, NB], [1, NB]]))
        numo = small.tile([128, DT, NB], F32, tag="numo")
        nc.vector.tensor_reduce(numo[:], wv2[:], mybir.AxisListType.X,
                                ALU.add)
        nc.vector.reciprocal(deno[:], deno[:])
        nc.vector.tensor_mul(oT[:], numo[:], deno[:])

        # ---- gating at rows 0 and 15 (fp16 operands for the matmuls) ----
        o01 = small.tile([128, DT, 2], F16)
        v01 = small.tile([128, DT, 2], F16)
        for dt in range(DT):
            nc.vector.tensor_copy(o01[:, dt, :],
                                  _ap(oT, dt * NB, [[NB - 1, 2]]))
            nc.vector.tensor_copy(v01[:, dt, :],
                                  _ap(vTf, dt * NB, [[NB - 1, 2]]))
        G01 = small.tile([128, DT, 2], F32)
        for mt in range(DT):
            pt = psum.tile([128, 2], F32, tag="mmps")
            for kt in range(DT):
                nc.tensor.matmul(
                    pt[:], wsb["gW1"][:, kt * D + mt * 128: kt * D + (mt + 1) * 128],
                    o01[:, kt, :], start=(kt == 0), stop=False)
            for kt in range(DT):
                nc.tensor.matmul(
                    pt[:], wsb["gW2"][:, kt * D + mt * 128: kt * D + (mt + 1) * 128],
                    v01[:, kt, :], start=False, stop=(kt == DT - 1))
            nc.scalar.activation(G01[:, mt, :], pt[:], AF.Sigmoid,
                                 bias=bsb["gb"][:, mt:mt + 1])
        e01 = small.tile([128, DT, 2], F32)
        for dt in range(DT):
            tmp = small.tile([128, 2], F32, tag="etmp")
            nc.vector.tensor_sub(tmp[:], o01[:, dt, :], v01[:, dt, :])
            nc.vector.tensor_mul(tmp[:], tmp[:], G01[:, dt, :])
            nc.vector.tensor_add(e01[:, dt, :], v01[:, dt, :], tmp[:])

        # ---- fusion for both candidate slices ----
        # slice A: cols 0:16 with E=e01[...,0]; slice B: cols 1008:1024, E=e01[...,1]
        EA = small.tile([128, DT, 2, 16], F16)   # [dt, slice, 16]
        for dt in range(DT):
            for s in range(2):
                nc.vector.tensor_copy(EA[:, dt, s, :],
                                      _ap(e01, dt * 2 + s, [[0, 16]]))
        outT = small.tile([128, DT, 32], F32)
        for wname, bname, func, dstname in (("fW1", "fb1", AF.Relu, "fus"),
                                            ("fW2", "fb2", AF.Sigmoid, "gf")):
            dst = small.tile([128, DT, 32], F32, tag=dstname)
            if dstname == "fus":
                fus = dst
            else:
                gf = dst
            for mt in range(DT):
                pt = psum.tile([128, 32], F32, tag="mmps")
                for kt in range(6):
                    if kt < 2:
                        rhs = _ap(inp, kt * T, [[T - 16, 2], [1, 16]])
                    elif kt < 4:
                        rhs = _ap(hT, (kt - 2) * T, [[T - 16, 2], [1, 16]])
                    else:
                        rhs = EA[:, kt - 4, :, :]
                    nc.tensor.matmul(
                        pt[:],
                        wsb[wname][:, kt * D + mt * 128: kt * D + (mt + 1) * 128],
                        rhs, start=(kt == 0), stop=(kt == 5))
                nc.scalar.activation(dst[:, mt, :], pt[:],
                                     func, bias=bsb[bname][:, mt:mt + 1])
        for mt in range(DT):
            xf = _ap(inp, mt * T, [[T - 16, 2], [1, 16]])
            of = outT[:, mt, :]
            nc.vector.tensor_sub(of, fus[:, mt, :], xf)
            nc.vector.tensor_mul(of, of, gf[:, mt, :])
            nc.vector.tensor_add(of, of, xf)
        nc.sync.dma_start(
            out=out_d.rearrange("(dt p) c -> p dt c", p=128),
            in_=outT[:])
    nc.compile()
    return nc


_NC = None


def _get_nc():
    global _NC
    if _NC is None:
        _NC = build_nc()
    return _NC


def _consts():
    il = np.arange(ICH)
    m = (il[None, :] > il[:, None]).astype(np.float16).reshape(-1)
    m16 = np.broadcast_to(m, (128, ICH * ICH)).copy()
    bi = np.arange(NB)
    blk = (bi[None, :] > bi[:, None]).astype(np.float32).reshape(-1)
    blkmask = np.broadcast_to(blk, (128, NB * NB)).copy()
    e64 = np.zeros(R, np.float16); e64[R - 1] = 1.0
    eps64 = np.broadcast_to(e64, (128, R)).copy()
    e16 = np.zeros(NB, np.float32); e16[NB - 1] = 1.0
    eps16 = np.broadcast_to(e16, (128, NB)).copy()
    selI = np.zeros((64, P), np.float16)
    selJ = np.zeros((64, P), np.float16)
    col = 0
    for c in range(NCH):                       # diagonal 4x4 blocks
        for il_ in range(ICH):
            for jl in range(ICH):
                selI[ICH * c + il_, col] = 1.0
                selJ[ICH * c + jl, col] = 1.0
                col += 1
    assert col == DIAGW
    for c in range(NCH - 1):                   # tails
        for il_ in range(ICH):
            for j in range(ICH * c + ICH, R):
                selI[ICH * c + il_, col] = 1.0
                selJ[j, col] = 1.0
                col += 1
    assert col == P
    sela = np.concatenate([selI, selJ], 0)
    return m16, blkmask, eps64, eps16, sela


def prep_in_maps(inputs):
    x = np.asarray(inputs["x"], np.float32)
    m16, blkmask, eps64, eps16, sela = _consts()
    wnames = ("fcW", "mW1", "mW2", "s2tW1", "s2tW", "gW1", "gW2", "fW1", "fW2")
    bnames = ("fcb", "mb", "s2tb1", "s2tb", "gb", "fb1", "fb2")

    in_maps = []
    for core in range(NCORES):
        b = core % B
        sfx = "_fw" if core < B else "_bw"
        xf = x[b].reshape(T, D)
        if core >= B:
            xf = xf[::-1]
        m = {"xT": np.ascontiguousarray(xf.T.astype(np.float16)),
             "m16": m16, "blkmask": blkmask,
             "eps64": eps64, "eps16": eps16, "sela": sela}
        for nm in wnames:
            m[nm] = np.ascontiguousarray(
                np.asarray(inputs[nm + sfx]).astype(np.float16))
        for nm in bnames:
            m[nm] = np.ascontiguousarray(
                np.asarray(inputs[nm + sfx]).astype(np.float32))
        in_maps.append(m)
    return in_maps


def assemble(outs):
    u_fw = np.stack([outs[b]["outT"][:, 0:16].T for b in range(B)])
    u_bw = np.stack([outs[B + b]["outT"][:, 16:32].T[::-1] for b in range(B)])
    return np.concatenate([u_fw, u_bw], axis=-1).astype(np.float32)


def kernel(**inputs):
    in_maps = prep_in_maps(inputs)
    res = bass_utils.run_bass_kernel_spmd(_get_nc(), in_maps,
                                          core_ids=list(range(NCORES)))
    return assemble(res.results)


# revision 10
# speedup vs baseline: 1.1057x; 1.0518x over previous
"""BiBloSAN Trainium2 kernel (v2).

Shapes: B=4, N=16 blocks, R=64 tokens/block, D=256.
Sharding: one (batch, direction) pair per core -> 8 cores, no collectives.
The bw direction runs the SAME SPMD program on a host-reversed token
sequence (flat reverse maps the j<i mask onto the j>i program exactly).

v2 layout: full fp16 datapath (weights + activations), tight pair packing
(2176 pair-columns per (block, feature-half) instead of 2560), stacked
xi/xj K=128 select-matmuls, and folds rebalanced across DVE + Pool.

Pair-column layout per (block, dt): P = 2176 columns of (i, j) pairs.
 - cols [0, 256): 16 diagonal 4x4 blocks (chunk-major); for chunk c,
   sub-col (il, jl) is pair (i=4c+il, j=4c+jl); entries jl<=il are
   invalid and are zeroed with the m16 mask after exp.
 - cols [256, 2176): tails; chunk c (c=0..14) holds rows i=4c+il,
   j in [4c+4, 64), width jw'=60-4c, at offset TOFF[c].
Numerator/denominator folds: in-place fp16 halving adds per tail chunk
down to width <= 8, then a TensorReduce; the 4-wide diagonal part is a
single strided TensorReduce over all 16 chunks.
"""

import numpy as np
from contextlib import ExitStack

import concourse.bass as bass
import concourse.mybir as mybir
import concourse.tile as tile
from concourse import bacc, bass_utils

F32 = mybir.dt.float32
F16 = mybir.dt.float16
AF = mybir.ActivationFunctionType
ALU = mybir.AluOpType

B, NB, R, D = 4, 16, 64, 256
T = NB * R          # 1024 tokens
DT = D // 128       # 2 partition tiles of feature dim
C = 5.0
NCORES = 8
GB = 4              # blocks per instruction group
NGRP = NB // GB
ICH = 8             # i-rows per chunk
NCH = R // ICH      # 8 chunks
DIAGW = NCH * ICH * ICH       # 512 diagonal pair-cols
TJW = [56 - 8 * c for c in range(NCH - 1)]   # tail widths per chunk
TOFF = []
_o = DIAGW
for _c in range(NCH - 1):
    TOFF.append(_o)
    _o += ICH * TJW[_c]
P = _o              # 2304 pair-cols per (block, dt)
assert P == 2304


def _ap(t, offset, dims):
    """Raw AP on sbuf tile t: dims = [[step, count], ...] free dims."""
    base = t[:]
    return bass.AP(tensor=base.tensor, offset=base.offset + offset,
                   ap=[list(base.ap[0])] + [list(d) for d in dims])


def build_nc():
    nc = bacc.Bacc("TRN2", target_bir_lowering=False, debug=False,
                   num_devices=NCORES)

    # ---- DRAM I/O ----
    xT_d = nc.dram_tensor("xT", [D, T], F16, kind="ExternalInput").ap()
    w_d = {}
    for nm in ("fcW", "mW1", "mW2", "s2tW1", "s2tW", "gW1", "gW2"):
        w_d[nm] = nc.dram_tensor(nm, [D, D], F16, kind="ExternalInput").ap()
    for nm in ("fW1", "fW2"):
        w_d[nm] = nc.dram_tensor(nm, [3 * D, D], F16, kind="ExternalInput").ap()
    b_d = {}
    for nm in ("fcb", "mb", "s2tb1", "s2tb", "gb", "fb1", "fb2"):
        b_d[nm] = nc.dram_tensor(nm, [D], F32, kind="ExternalInput").ap()
    sela_d = nc.dram_tensor("sela", [128, P], F16, kind="ExternalInput").ap()
    m16_d = nc.dram_tensor("m16", [128, ICH * ICH], F16,
                           kind="ExternalInput").ap()
    eps64_d = nc.dram_tensor("eps64", [128, R], F16, kind="ExternalInput").ap()
    blkm_d = nc.dram_tensor("blkmask", [128, NB * NB], F32,
                            kind="ExternalInput").ap()
    eps16_d = nc.dram_tensor("eps16", [128, NB], F32, kind="ExternalInput").ap()
    out_d = nc.dram_tensor("outT", [D, 32], F32, kind="ExternalOutput").ap()

    with tile.TileContext(nc) as tc, ExitStack() as ctx:
        const = ctx.enter_context(tc.tile_pool(name="const", bufs=1))
        big = ctx.enter_context(tc.tile_pool(name="big", bufs=1))
        work = ctx.enter_context(tc.tile_pool(name="work", bufs=2))
        wpool = ctx.enter_context(tc.tile_pool(name="wpool", bufs=3))
        small = ctx.enter_context(tc.tile_pool(name="small", bufs=4))
        xijsb_pool = ctx.enter_context(tc.tile_pool(name="xijsb", bufs=6))
        psum = ctx.enter_context(
            tc.tile_pool(name="psum", bufs=1, space="PSUM"))
        xijps_pool = ctx.enter_context(
            tc.tile_pool(name="xijps", bufs=1, space="PSUM"))
        pairps_pool = ctx.enter_context(
            tc.tile_pool(name="pairps", bufs=2, space="PSUM"))

        # ---- load weights / constants (one DMA per tensor) ----
        wsb = {}
        def load_w(nm, nkt=2):
            t = const.tile([128, nkt * D], F16, tag=nm)
            nc.sync.dma_start(
                out=t[:].rearrange("p (kt e) -> p kt e", kt=nkt),
                in_=w_d[nm].rearrange("(kt p) e -> p kt e", p=128))
            wsb[nm] = t
        bsb = {}
        def load_b(nm):
            t = const.tile([128, DT], F32, tag=nm)
            nc.sync.dma_start(out=t[:],
                              in_=b_d[nm].rearrange("(dt p) -> p dt", p=128))
            bsb[nm] = t
        load_w("fcW")
        load_b("fcb")
        xT = big.tile([128, DT, T], F16, tag="xT")
        for hf in range(2):
            for dt in range(DT):
                nc.sync.dma_start(
                    out=xT[:, dt, hf * 512:(hf + 1) * 512],
                    in_=xT_d[dt * 128:(dt + 1) * 128, hf * 512:(hf + 1) * 512])
        for nm in ("mW1", "mW2"):
            load_w(nm)
        load_b("mb")
        mbC = const.tile([128, DT], F32)
        nc.scalar.mul(mbC[:], bsb["mb"][:], 1.0 / C)
        sela = const.tile([128, P], F16)
        nc.sync.dma_start(out=sela[:], in_=sela_d[:, :])
        m16 = const.tile([128, ICH * ICH], F16)
        nc.sync.dma_start(out=m16[:], in_=m16_d[:, :])
        eps64 = const.tile([128, R], F16)
        nc.sync.dma_start(out=eps64[:], in_=eps64_d[:, :])
        for nm in ("s2tb1", "s2tb", "gb", "fb1", "fb2"):
            load_b(nm)
        blkm = const.tile([128, NB * NB], F32)
        nc.sync.dma_start(out=blkm[:], in_=blkm_d[:, :])
        eps16 = const.tile([128, NB], F32)
        nc.sync.dma_start(out=eps16[:], in_=eps16_d[:, :])
        for nm in ("s2tW1", "s2tW", "gW1", "gW2"):
            load_w(nm)
        for nm in ("fW1", "fW2"):
            load_w(nm, nkt=6)

        # ---- FC: inp = relu(fcW.T @ xT + fcb), fp16 out ----
        inp = big.tile([128, DT, T], F16)
        for ncs in range(0, T, 512):
            for mt in range(DT):
                pt = psum.tile([128, 512], F32, tag="mmps")
                for kt in range(DT):
                    nc.tensor.matmul(
                        pt[:],
                        wsb["fcW"][:, kt * D + mt * 128: kt * D + (mt + 1) * 128],
                        xT[:, kt, ncs:ncs + 512],
                        start=(kt == 0), stop=(kt == DT - 1))
                nc.vector.tensor_scalar(inp[:, mt, ncs:ncs + 512], pt[:],
                                        bsb["fcb"][:, mt:mt + 1], 0.0,
                                        ALU.add, ALU.max)

        # ---- intra-block mSA ----
        ND = big.tile([128, DT, 2, T], F16, tag="ND")  # [...,0,:]=den [...,1,:]=num
        hT = big.tile([128, DT, T], F16, tag="hT")
        fT = big.tile([128, DT, T], F16, tag="fT")
        eT = big.tile([128, DT, T], F16, tag="eT")
        SUMS = small.tile([128, DT, NB], F32)
        NUMV = small.tile([128, DT, NB], F32)
        vT = small.tile([128, DT, NB], F16)
        rS = small.tile([128, DT, NB], F32, tag="rS")
        viT = small.tile([128, DT, NB], F32)
        vjT = small.tile([128, DT, NB], F32)

        # pair-matmul chunking of the P columns (psum-bank-sized windows)
        PCH = [(0, 1152), (1152, 1152)]

        def msa_group(g):
            # xi/xj for the 4 blocks, stacked on partitions (0-63 xi, 64-127 xj)
            xijs = []
            for bg in range(GB):
                tok0 = (g * GB + bg) * R
                xp = xijps_pool.tile([128, D], F32, tag="xijps")
                for kt in range(DT):
                    nc.tensor.matmul(
                        xp[0:64, :], inp[:, kt, tok0:tok0 + 64],
                        wsb["mW1"][:, kt * D:(kt + 1) * D],
                        start=(kt == 0), stop=(kt == DT - 1))
                for kt in range(DT):
                    nc.tensor.matmul(
                        xp[64:128, :], inp[:, kt, tok0:tok0 + 64],
                        wsb["mW2"][:, kt * D:(kt + 1) * D],
                        start=(kt == 0), stop=(kt == DT - 1))
                xsb = xijsb_pool.tile([128, D], F16, tag="xijsb")
                nc.scalar.copy(xsb[:], xp[:])
                xijs.append(xsb)

            for dt in range(DT):
                w16 = wpool.tile([128, GB, 2, P], F16, tag="w16")
                for bg in range(GB):
                    lhs = xijs[bg][:, dt * 128:(dt + 1) * 128]
                    for c0, cw in PCH:
                        pt = pairps_pool.tile([128, 1152], F32, tag="pairps")
                        for w0 in range(0, cw, 512):
                            ww = min(512, cw - w0)
                            nc.tensor.matmul(
                                pt[:, w0:w0 + ww], lhs,
                                sela[:, c0 + w0: c0 + w0 + ww],
                                start=True, stop=True)
                        nc.scalar.activation(
                            w16[:, bg, 0, c0:c0 + cw], pt[:, :cw], AF.Tanh,
                            bias=mbC[:, dt:dt + 1], scale=1.0 / C)
                # exp in place over the w halves, per block
                for bg in range(GB):
                    wall = _ap(w16, bg * 2 * P, [[1, P]])
                    nc.scalar.activation(wall, wall, AF.Exp, scale=C)

                # diagonal region: mask, then wx (Pool, per 2 blocks),
                # then halving folds (DVE)
                for bp in range(2):
                    o2 = bp * 2 * 2 * P
                    dmw = _ap(w16, o2, [[2 * P, 2], [ICH * ICH, NCH], [1, ICH * ICH]])
                    dm = _ap(m16, 0, [[0, 2], [0, NCH], [1, ICH * ICH]])
                    nc.gpsimd.tensor_mul(dmw, dmw, dm)
                    dwx = _ap(w16, o2 + P,
                              [[2 * P, 2], [ICH * ICH, NCH], [ICH, ICH], [1, ICH]])
                    dw = _ap(w16, o2,
                             [[2 * P, 2], [ICH * ICH, NCH], [ICH, ICH], [1, ICH]])
                    dx = _ap(inp, dt * T + (g * GB + bp * 2) * R,
                             [[R, 2], [ICH, NCH], [0, ICH], [1, ICH]])
                    nc.gpsimd.tensor_mul(dwx, dw, dx)
                # diag fold: 8 -> 4 -> 2 -> 1 (into ND)
                da0 = _ap(w16, 0, [[2 * P, GB], [P, 2], [ICH, R], [1, 4]])
                da1 = _ap(w16, 4, [[2 * P, GB], [P, 2], [ICH, R], [1, 4]])
                nc.vector.tensor_add(da0, da0, da1)
                db0 = _ap(w16, 0, [[2 * P, GB], [P, 2], [ICH, R], [1, 2]])
                db1 = _ap(w16, 2, [[2 * P, GB], [P, 2], [ICH, R], [1, 2]])
                nc.vector.tensor_add(db0, db0, db1)
                ndo = _ap(ND, dt * 2 * T + g * GB * R,
                          [[R, GB], [T, 2], [1, R]])
                dc0 = _ap(w16, 0, [[2 * P, GB], [P, 2], [ICH, R]])
                dc1 = _ap(w16, 1, [[2 * P, GB], [P, 2], [ICH, R]])
                nc.vector.tensor_add(ndo, dc0, dc1)

                # tails: wx mul, uneven in-place halving folds into TSUM
                TSUM = small.tile([128, GB, 2, R], F16, tag="tsum")
                for c in range(NCH - 1):
                    jwp = TJW[c]
                    toff = TOFF[c]
                    wv = _ap(w16, toff, [[2 * P, GB], [jwp, ICH], [1, jwp]])
                    wxv = _ap(w16, P + toff,
                              [[2 * P, GB], [jwp, ICH], [1, jwp]])
                    xv = _ap(inp, dt * T + g * GB * R + ICH * c + ICH,
                             [[R, GB], [0, ICH], [1, jwp]])
                    nc.vector.tensor_mul(wxv, wv, xv)
                    h = jwp
                    first = True
                    while h > 2:
                        k = h // 2
                        hp = h - k      # ceil
                        a0 = _ap(w16, toff,
                                 [[2 * P, GB], [P, 2], [jwp, ICH], [1, k]])
                        a1 = _ap(w16, toff + hp,
                                 [[2 * P, GB], [P, 2], [jwp, ICH], [1, k]])
                        if first and c == 0:
                            nc.gpsimd.tensor_add(a0, a0, a1)
                        else:
                            nc.vector.tensor_add(a0, a0, a1)
                        first = False
                        h = hp
                    tout = _ap(TSUM, ICH * c,
                               [[2 * R, GB], [R, 2], [1, ICH]])
                    t0 = _ap(w16, toff, [[2 * P, GB], [P, 2], [jwp, ICH]])
                    t1 = _ap(w16, toff + 1, [[2 * P, GB], [P, 2], [jwp, ICH]])
                    nc.vector.tensor_add(tout, t0, t1)
                # ND rows 0..55 += TSUM; den += eps (row 63)
                nda = _ap(ND, dt * 2 * T + g * GB * R,
                          [[R, GB], [T, 2], [1, R - ICH]])
                tsa = _ap(TSUM, 0, [[2 * R, GB], [R, 2], [1, R - ICH]])
                nc.vector.tensor_add(nda, nda, tsa)
                epsa = _ap(ND, dt * 2 * T + g * GB * R, [[R, GB], [1, R]])
                nc.vector.tensor_add(epsa, epsa,
                                     _ap(eps64, 0, [[0, GB], [1, R]]))

                # h = num / den  (fp16)
                g0 = g * GB * R
                den = ND[:, dt, 0, g0:g0 + GB * R]
                num = ND[:, dt, 1, g0:g0 + GB * R]
                with nc.allow_low_precision("fp16 softmax normalize"):
                    nc.vector.reciprocal(den, den)
                nc.vector.tensor_mul(hT[:, dt, g0:g0 + GB * R], num, den)

        def s2t_group(g):
            # ---- s2t for group g's 4 blocks (runs one group behind) ----
            GC = GB * R
            g0 = g * GC
            for mt in range(DT):
                ptf = psum.tile([128, GC], F32, tag="mmps")
                for kt in range(DT):
                    nc.tensor.matmul(
                        ptf[:],
                        wsb["s2tW1"][:, kt * D + mt * 128: kt * D + (mt + 1) * 128],
                        hT[:, kt, g0:g0 + GC], start=(kt == 0),
                        stop=(kt == DT - 1))
                nc.vector.tensor_scalar(fT[:, mt, g0:g0 + GC], ptf[:],
                                        bsb["s2tb1"][:, mt:mt + 1], 0.0,
                                        ALU.add, ALU.max)
            for mt in range(DT):
                pte = psum.tile([128, GC], F32, tag="mmps")
                for kt in range(DT):
                    nc.tensor.matmul(
                        pte[:],
                        wsb["s2tW"][:, kt * D + mt * 128: kt * D + (mt + 1) * 128],
                        fT[:, kt, g0:g0 + GC], start=(kt == 0),
                        stop=(kt == DT - 1))
                nc.scalar.activation(eT[:, mt, g0:g0 + GC], pte[:], AF.Exp,
                                     bias=bsb["s2tb"][:, mt:mt + 1])
            for dt in range(DT):
                nc.vector.tensor_reduce(
                    SUMS[:, dt, g * GB:(g + 1) * GB],
                    eT[:, dt, g0:g0 + GC].rearrange("p (n r) -> p n r", r=R),
                    mybir.AxisListType.X, ALU.add)
                wh = work.tile([128, GC], F16, tag="wh")
                nc.vector.tensor_mul(wh[:], eT[:, dt, g0:g0 + GC],
                                     hT[:, dt, g0:g0 + GC])
                nc.vector.tensor_reduce(
                    NUMV[:, dt, g * GB:(g + 1) * GB],
                    wh[:].rearrange("p (n r) -> p n r", r=R),
                    mybir.AxisListType.X, ALU.add)
            gB = g * GB
            for dt in range(DT):
                nc.vector.reciprocal(rS[:, dt, gB:gB + GB],
                                     SUMS[:, dt, gB:gB + GB])
                nc.vector.tensor_mul(vT[:, dt, gB:gB + GB],
                                     NUMV[:, dt, gB:gB + GB],
                                     rS[:, dt, gB:gB + GB])
            for dst, wname in ((viT, "mW1"), (vjT, "mW2")):
                w = wsb[wname]
                for mt in range(DT):
                    pt = psum.tile([128, GB], F32, tag="mmps")
                    for kt in range(DT):
                        nc.tensor.matmul(
                            pt[:],
                            w[:, kt * D + mt * 128: kt * D + (mt + 1) * 128],
                            vT[:, kt, gB:gB + GB], start=(kt == 0),
                            stop=(kt == DT - 1))
                    nc.vector.tensor_copy(dst[:, mt, gB:gB + GB], pt[:])

        for g in range(NGRP):
            msa_group(g)
            if g >= 1:
                s2t_group(g - 1)
        s2t_group(NGRP - 1)

        oT = small.tile([128, DT, NB], F32)
        ub = work.tile([128, DT, NB, NB], F32, tag="ublk")
        vi2 = _ap(viT, 0, [[NB, DT], [1, NB], [0, NB]])
        vj2 = _ap(vjT, 0, [[NB, DT], [0, NB], [1, NB]])
        nc.vector.tensor_add(ub[:], vi2, vj2)
        for dt in range(DT):
            nc.scalar.activation(ub[:, dt], ub[:, dt], AF.Tanh,
                                 bias=mbC[:, dt:dt + 1], scale=1.0 / C)
        nc.scalar.activation(ub[:], ub[:], AF.Exp, scale=C)
        bm = _ap(blkm, 0, [[0, DT], [NB, NB], [1, NB]])
        nc.vector.tensor_mul(ub[:], ub[:], bm)
        deno = small.tile([128, DT, NB], F32, tag="deno")
        nc.vector.tensor_reduce(deno[:], ub[:], mybir.AxisListType.X,
                                ALU.add)
        nc.vector.tensor_add(deno[:], deno[:],
                             _ap(eps16, 0, [[0, DT], [1, NB]]))
        wv2 = work.tile([128, DT, NB, NB], F32, tag="wv2")
        vTf = small.tile([128, DT, NB], F32, tag="vTf")
        for dt in range(DT):
            nc.vector.tensor_copy(vTf[:, dt, :], vT[:, dt, :])
        nc.vector.tensor_mul(wv2[:], ub[:],
                             _ap(vTf, 0, [[NB, DT], [0, NB], [1, NB]]))
        numo = small.tile([128, DT, NB], F32, tag="numo")
        nc.vector.tensor_reduce(numo[:], wv2[:], mybir.AxisListType.X,
                                ALU.add)
        nc.vector.reciprocal(deno[:], deno[:])
        nc.vector.tensor_mul(oT[:], numo[:], deno[:])

        # ---- gating at rows 0 and 15 (fp16 operands for the matmuls) ----
        o01 = small.tile([128, DT, 2], F16)
        v01 = small.tile([128, DT, 2], F16)
        for dt in range(DT):
            nc.vector.tensor_copy(o01[:, dt, :],
                                  _ap(oT, dt * NB, [[NB - 1, 2]]))
            nc.vector.tensor_copy(v01[:, dt, :],
                                  _ap(vTf, dt * NB, [[NB - 1, 2]]))
        G01 = small.tile([128, DT, 2], F32)
        for mt in range(DT):
            pt = psum.tile([128, 2], F32, tag="mmps")
            for kt in range(DT):
                nc.tensor.matmul(
                    pt[:], wsb["gW1"][:, kt * D + mt * 128: kt * D + (mt + 1) * 128],
                    o01[:, kt, :], start=(kt == 0), stop=False)
            for kt in range(DT):
                nc.tensor.matmul(
                    pt[:], wsb["gW2"][:, kt * D + mt * 128: kt * D + (mt + 1) * 128],
                    v01[:, kt, :], start=False, stop=(kt == DT - 1))
            nc.scalar.activation(G01[:, mt, :], pt[:], AF.Sigmoid,
                                 bias=bsb["gb"][:, mt:mt + 1])
        e01 = small.tile([128, DT, 2], F32)
        for dt in range(DT):
            tmp = small.tile([128, 2], F32, tag="etmp")
            nc.vector.tensor_sub(tmp[:], o01[:, dt, :], v01[:, dt, :])
            nc.vector.tensor_mul(tmp[:], tmp[:], G01[:, dt, :])
            nc.vector.tensor_add(e01[:, dt, :], v01[:, dt, :], tmp[:])

        # ---- fusion for both candidate slices ----
        # slice A: cols 0:16 with E=e01[...,0]; slice B: cols 1008:1024, E=e01[...,1]
        EA = small.tile([128, DT, 2, 16], F16)   # [dt, slice, 16]
        for dt in range(DT):
            for s in range(2):
                nc.vector.tensor_copy(EA[:, dt, s, :],
                                      _ap(e01, dt * 2 + s, [[0, 16]]))
        outT = small.tile([128, DT, 32], F32)
        for wname, bname, func, dstname in (("fW1", "fb1", AF.Relu, "fus"),
                                            ("fW2", "fb2", AF.Sigmoid, "gf")):
            dst = small.tile([128, DT, 32], F32, tag=dstname)
            if dstname == "fus":
                fus = dst
            else:
                gf = dst
            for mt in range(DT):
                pt = psum.tile([128, 32], F32, tag="mmps")
                for kt in range(6):
                    if kt < 2:
                        rhs = _ap(inp, kt * T, [[T - 16, 2], [1, 16]])
                    elif kt < 4:
                        rhs = _ap(hT, (kt - 2) * T, [[T - 16, 2], [1, 16]])
                    else:
                        rhs = EA[:, kt - 4, :, :]
                    nc.tensor.matmul(
                        pt[:],
                        wsb[wname][:, kt * D + mt * 128: kt * D + (mt + 1) * 128],
                        rhs, start=(kt == 0), stop=(kt == 5))
                nc.scalar.activation(dst[:, mt, :], pt[:],
                                     func, bias=bsb[bname][:, mt:mt + 1])
        for mt in range(DT):
            xf = _ap(inp, mt * T, [[T - 16, 2], [1, 16]])
            of = outT[:, mt, :]
            nc.vector.tensor_sub(of, fus[:, mt, :], xf)
            nc.vector.tensor_mul(of, of, gf[:, mt, :])
            nc.vector.tensor_add(of, of, xf)
        nc.sync.dma_start(
            out=out_d.rearrange("(dt p) c -> p dt c", p=128),
            in_=outT[:])
    nc.compile()
    return nc


_NC = None


def _get_nc():
    global _NC
    if _NC is None:
        _NC = build_nc()
    return _NC


def _consts():
    il = np.arange(ICH)
    m = (il[None, :] > il[:, None]).astype(np.float16).reshape(-1)
    m16 = np.broadcast_to(m, (128, ICH * ICH)).copy()
    bi = np.arange(NB)
    blk = (bi[None, :] > bi[:, None]).astype(np.float32).reshape(-1)
    blkmask = np.broadcast_to(blk, (128, NB * NB)).copy()
    e64 = np.zeros(R, np.float16); e64[R - 1] = 1.0
    eps64 = np.broadcast_to(e64, (128, R)).copy()
    e16 = np.zeros(NB, np.float32); e16[NB - 1] = 1.0
    eps16 = np.broadcast_to(e16, (128, NB)).copy()
    selI = np.zeros((64, P), np.float16)
    selJ = np.zeros((64, P), np.float16)
    col = 0
    for c in range(NCH):                       # diagonal 4x4 blocks
        for il_ in range(ICH):
            for jl in range(ICH):
                selI[ICH * c + il_, col] = 1.0
                selJ[ICH * c + jl, col] = 1.0
                col += 1
    assert col == DIAGW
    for c in range(NCH - 1):                   # tails
        for il_ in range(ICH):
            for j in range(ICH * c + ICH, R):
                selI[ICH * c + il_, col] = 1.0
                selJ[j, col] = 1.0
                col += 1
    assert col == P
    sela = np.concatenate([selI, selJ], 0)
    return m16, blkmask, eps64, eps16, sela


def prep_in_maps(inputs):
    x = np.asarray(inputs["x"], np.float32)
    m16, blkmask, eps64, eps16, sela = _consts()
    wnames = ("fcW", "mW1", "mW2", "s2tW1", "s2tW", "gW1", "gW2", "fW1", "fW2")
    bnames = ("fcb", "mb", "s2tb1", "s2tb", "gb", "fb1", "fb2")

    in_maps = []
    for core in range(NCORES):
        b = core % B
        sfx = "_fw" if core < B else "_bw"
        xf = x[b].reshape(T, D)
        if core >= B:
            xf = xf[::-1]
        m = {"xT": np.ascontiguousarray(xf.T.astype(np.float16)),
             "m16": m16, "blkmask": blkmask,
             "eps64": eps64, "eps16": eps16, "sela": sela}
        for nm in wnames:
            m[nm] = np.ascontiguousarray(
                np.asarray(inputs[nm + sfx]).astype(np.float16))
        for nm in bnames:
            m[nm] = np.ascontiguousarray(
                np.asarray(inputs[nm + sfx]).astype(np.float32))
        in_maps.append(m)
    return in_maps


def assemble(outs):
    u_fw = np.stack([outs[b]["outT"][:, 0:16].T for b in range(B)])
    u_bw = np.stack([outs[B + b]["outT"][:, 16:32].T[::-1] for b in range(B)])
    return np.concatenate([u_fw, u_bw], axis=-1).astype(np.float32)


def kernel(**inputs):
    in_maps = prep_in_maps(inputs)
    res = bass_utils.run_bass_kernel_spmd(_get_nc(), in_maps,
                                          core_ids=list(range(NCORES)))
    return assemble(res.results)
